# revision 23
# baseline (speedup 1.0000x reference)
"""Low-rank self-attention Trainium2 kernel — tunnel-optimized split.

The axon tunnel to the NeuronCores moves ~30-100 MB/s with ~0.1s fixed
cost per transfer, while the rank-32 projections are ~4 GFLOP of host BLAS
(~0.1s). So the host computes QKV = x@Wqkv+b (f32) and the final
attn@Wo+bo projection, and the device runs only the S^2 attention core
(the dominant FLOPs): scoresT = K^T.T @ Q^T (rank-32 contraction, 4-way
row-packed fp32r), expS = exp(scale*scores) on ACT, and attn^T[33,q]
accumulated over 32 k-tiles with a fused ones-column denominator.

Sharding: one batch per core on 4 of the 8 cores — full queries per core,
so K/V are never duplicated across cores (pairs of query-half cores would
each need the full K/V). Per-core traffic: one packed bf16 input blob
(Q^T/K^T [32,4096] + V [4096,32] = 768KB) and one packed bf16 output
(attn^T [32,4096] + den f32-as-bf16-pairs = 272KB).

The jitted shard_map callable is built once and cached; donated output
buffers are created on-device (zeros_fn).
"""
import os
import sys

sys.path.insert(0, "/opt/trn_rl_repo")

import numpy as np
import ml_dtypes

import concourse.bass as bass
import concourse.mybir as mybir
import concourse.tile as tile
from bass_rust import ScopedClock

BF16 = mybir.dt.bfloat16
F32 = mybir.dt.float32
F32R = mybir.dt.float32r

B, S, D, R = 4, 4096, 1024, 32
N_CORES = 4
SCALE = float(R) ** -0.5

QT_OFF = 0
KT_OFF = QT_OFF + R * S           # 131072
V_OFF = KT_OFF + R * S            # 262144 (V natural [S, R])
BLOB = V_OFF + S * R              # 393216 bf16 elems (768 KiB)
Y_ATTN = R * S                    # 131072
Y_ELEMS = Y_ATTN + 2 * S          # + den as f32 bitcast into bf16 pairs

_DBG = bool(os.environ.get("KERNEL_DEBUG_TIMING"))


class ChunkedDrainTileContext(tile.TileContext):
    """This walrus build rejects >1 sync wait on the kernel-tail drain;
    spread the final drain's waits across single-wait SP nops."""

    def _drain_and_barrier(self, tick_clock, wait_clock):
        nc = self.nc
        MAX_NOPS = 40
        nops = [nc.sync.nop(nofuse=True) for _ in range(MAX_NOPS)]
        drain_inst = nc.sync.drain()
        wait_clock.add_sem_waits(
            drain_inst.ins, ScopedClock({None: tick_clock.global_clock})
        )
        si = drain_inst.ins.sync_info
        waits = list(si.on_wait) if si and si.on_wait else []
        if len(waits) > 1:
            assert len(waits) <= 1 + MAX_NOPS, f"too many drain waits: {len(waits)}"
            drain_inst.ins.sync_info = mybir.SyncInfo(
                on_wait=[waits[0]], on_update=si.on_update
            )
            for i, w in enumerate(waits[1:]):
                nop = nops[i]
                old = nop.ins.sync_info
                nop.ins.sync_info = mybir.SyncInfo(
                    on_wait=[w], on_update=old.on_update if old else []
                )
        nc.all_engine_barrier()
        assert self.sems is not None
        popped = nc._tile_sem_poison_stack.pop()
        assert popped is self._sem_poison
        nc.clear_and_free_semaphores(list(self.sems.allocated().values()))
        nc.all_engine_barrier()
        split_multi_waits(nc)


def split_multi_waits(nc):
    """walrus in this container rejects instructions with more than one sync
    wait; split extras onto same-engine nops placed immediately before."""
    for f in nc.m.functions:
        for bb in f.blocks:
            snap = list(bb.instructions)
            if not any(
                inst.sync_info and inst.sync_info.on_wait
                and len(inst.sync_info.on_wait) > 1
                for inst in snap
            ):
                continue
            newlist = []
            created = set()
            for inst in snap:
                si = inst.sync_info
                waits = list(si.on_wait) if si and si.on_wait else []
                if len(waits) > 1:
                    eng = inst.engine
                    for w in waits[:-1]:
                        nop = nc.engines[eng].nop(nofuse=True)
                        nop.ins.sync_info = mybir.SyncInfo(
                            on_wait=[w], on_update=[]
                        )
                        created.add(nop.ins.name)
                        newlist.append(nop.ins)
                    inst.sync_info = mybir.SyncInfo(
                        on_wait=[waits[-1]], on_update=si.on_update
                    )
                newlist.append(inst)
            # nops were auto-appended to the current bb; strip strays
            for f2 in nc.m.functions:
                for bb2 in f2.blocks:
                    if bb2 is bb:
                        continue
                    cur = list(bb2.instructions)
                    if any(i.name in created for i in cur):
                        bb2.instructions = [
                            i for i in cur if i.name not in created
                        ]
            tail = [i for i in bb.instructions if i.name in created
                    and i not in snap]
            seen = set()
            final = []
            for i in newlist:
                if i.name in seen:
                    continue
                seen.add(i.name)
                final.append(i)
            bb.instructions = final


def build_kernel():
    nc = bass.Bass("TRN2", target_bir_lowering=False, debug=False)

    blob = nc.dram_tensor("blob", [BLOB], BF16, kind="ExternalInput")
    y = nc.dram_tensor("y", [Y_ELEMS], BF16, kind="ExternalOutput")

    NKT = S // 128   # 32 k-tiles
    NQC = S // 512   # 8 query chunks
    Exp = mybir.ActivationFunctionType.Exp

    with ChunkedDrainTileContext(nc) as tc:
        with (
            tc.tile_pool(name="persist", bufs=1) as pp,
            tc.tile_pool(name="work", bufs=3) as wp,
            tc.tile_pool(name="expp", bufs=2) as ep,
            tc.tile_pool(name="psB", bufs=1, space="PSUM") as psB,
            tc.tile_pool(name="psB2", bufs=2, space="PSUM") as psB2,
        ):
            qT16 = pp.tile([R, S], BF16)
            nc.sync.dma_start(
                qT16[:],
                blob.ap()[QT_OFF:QT_OFF + R * S].rearrange("(r s) -> r s", s=S),
            )
            kT16 = pp.tile([R, S], BF16)
            nc.sync.dma_start(
                kT16[:],
                blob.ap()[KT_OFF:KT_OFF + R * S].rearrange("(r s) -> r s", s=S),
            )
            vone = pp.tile([128, NKT, 33], BF16)
            nc.sync.dma_start(
                vone[:, :, 0:32],
                blob.ap()[V_OFF:V_OFF + S * R].rearrange(
                    "(kt p r) -> p kt r", p=128, r=R
                ),
            )
            nc.vector.memset(vone[:, :, 32], 1.0)

            qTf = pp.tile([R, S], F32R)
            nc.vector.tensor_copy(out=qTf[:], in_=qT16[:])
            kTf = pp.tile([R, S], F32R)
            nc.vector.tensor_copy(out=kTf[:], in_=kT16[:])
            qT_rep = pp.tile([128, S], F32R)
            kT_rep = pp.tile([128, S], F32R)
            for i in range(4):
                nc.sync.dma_start(qT_rep[32 * i:32 * i + 32, :], qTf[:])
                nc.sync.dma_start(kT_rep[32 * i:32 * i + 32, :], kTf[:])

            y2d = y.ap()[0:Y_ATTN].rearrange("(r s) -> r s", s=S)
            for qc in range(NQC):
                expT = ep.tile([128, NKT, 512], BF16, tag="expT")
                for g in range(NKT // 4):
                    ps_s = psB.tile([128, 4, 512], F32, tag="ps_s")
                    for i in range(4):
                        kt = g * 4 + i
                        nc.tensor.matmul(
                            ps_s[:, i, :],
                            kT_rep[32 * i:32 * i + 32,
                                   kt * 128:(kt + 1) * 128],
                            qT_rep[32 * i:32 * i + 32,
                                   qc * 512:(qc + 1) * 512],
                            start=True, stop=True,
                            skip_group_check=True,
                            tile_position=(32 * i, 0),
                        )
                    nc.scalar.activation(
                        expT[:, g * 4:(g + 1) * 4, :], ps_s[:], Exp,
                        scale=SCALE,
                    )
                pa = psB2.tile([128, 512], F32, tag="pa")
                for kt in range(NKT):
                    nc.tensor.matmul(
                        pa[0:33, :], vone[:, kt, :], expT[:, kt, :],
                        start=(kt == 0), stop=(kt == NKT - 1),
                    )
                attn16 = wp.tile([R, 512], BF16, tag="attn16")
                nc.vector.tensor_copy(out=attn16[:], in_=pa[0:32, :])
                den32 = wp.tile([1, 512], F32, tag="den32")
                nc.vector.tensor_copy(out=den32[:], in_=pa[32:33, :])
                nc.sync.dma_start(
                    y2d[:, qc * 512:(qc + 1) * 512], attn16[:]
                )
                nc.sync.dma_start(
                    y.ap()[Y_ATTN + qc * 1024:Y_ATTN + (qc + 1) * 1024]
                    .rearrange("(a b) -> a b", a=1),
                    den32[:].bitcast(BF16),
                )
    return nc


_CACHE = {}


def _get_runner():
    if "runner" in _CACHE:
        return _CACHE["runner"]
    import time as _time

    t0 = _time.time()
    import jax
    import jax.numpy as jnp
    from jax.experimental.shard_map import shard_map
    from jax.sharding import Mesh, NamedSharding, PartitionSpec

    from concourse import bass2jax

    bass2jax.install_neuronx_cc_hook()
    nc = build_kernel()

    out_aval = jax.core.ShapedArray((Y_ELEMS,), ml_dtypes.bfloat16)
    partition_name = (
        nc.partition_id_tensor.name if nc.partition_id_tensor else None
    )
    in_names = ("blob", "y") + ((partition_name,) if partition_name else ())

    def _body(blob_arg, yzero):
        operands = [blob_arg, yzero]
        if partition_name is not None:
            operands.append(bass2jax.partition_id_tensor())
        outs = bass2jax._bass_exec_p.bind(
            *operands,
            out_avals=(out_aval,),
            in_names=in_names,
            out_names=("y",),
            lowering_input_output_aliases=(),
            sim_require_finite=True,
            sim_require_nnan=True,
            nc=nc,
        )
        return tuple(outs)

    devices = jax.devices()[:N_CORES]
    mesh = Mesh(np.asarray(devices), ("core",))
    p = PartitionSpec("core")
    sharded = jax.jit(
        shard_map(_body, mesh=mesh, in_specs=(p, p), out_specs=(p,),
                  check_rep=False),
        donate_argnums=(1,), keep_unused=True,
    )
    zsh = NamedSharding(mesh, p)
    zeros_fn = jax.jit(
        lambda: jnp.zeros((N_CORES * Y_ELEMS,), ml_dtypes.bfloat16),
        out_shardings=zsh,
    )
    runner = (sharded, zeros_fn)
    _CACHE["runner"] = runner
    if _DBG:
        print(f"[kernel] runner built in {_time.time()-t0:.1f}s",
              file=sys.stderr)
    return runner


def kernel(x, Wq, bq, Wk, bk, Wv, bv, Wo, bo):
    import time as _time

    t0 = _time.time()
    sharded, zeros_fn = _get_runner()
    t1 = _time.time()
    yzero = zeros_fn()  # async; on-device, donated below

    bf16 = ml_dtypes.bfloat16
    x = np.ascontiguousarray(np.asarray(x, np.float32)).reshape(B * S, D)
    wqkv = np.concatenate(
        [np.asarray(Wq, np.float32), np.asarray(Wk, np.float32),
         np.asarray(Wv, np.float32)], axis=1)            # [D, 96]
    bqkv = np.concatenate(
        [np.asarray(bq, np.float32), np.asarray(bk, np.float32),
         np.asarray(bv, np.float32)])                    # [96]
    qkv = np.empty((B * S, 3 * R), np.float32)
    np.dot(x, wqkv, out=qkv)
    qkv += bqkv
    qkv16 = qkv.astype(bf16)                             # [B*S, 96]

    blob = np.empty((N_CORES, BLOB), bf16)
    for c in range(N_CORES):
        rows = slice(c * S, (c + 1) * S)
        blob[c, QT_OFF:QT_OFF + R * S] = qkv16[rows, 0:R].T.reshape(-1)
        blob[c, KT_OFF:KT_OFF + R * S] = qkv16[rows, R:2 * R].T.reshape(-1)
        blob[c, V_OFF:V_OFF + S * R] = qkv16[rows, 2 * R:3 * R].reshape(-1)
    t2 = _time.time()

    out_arrs = sharded(blob.reshape(-1), yzero)
    shards = sorted(out_arrs[0].addressable_shards,
                    key=lambda s: s.index[0].start or 0)
    datas = [s.data for s in shards]
    for d in datas:
        d.copy_to_host_async()                           # parallel D2H
    res = [np.asarray(d) for d in datas]                 # [Y_ELEMS] x cores
    t3 = _time.time()

    attn_rT = np.empty((R + 1, B * S), np.float32)       # rank-major + ones
    den_flat = np.empty((B * S,), np.float32)
    for c in range(N_CORES):
        rc = res[c]
        attn_rT[:R, c * S:(c + 1) * S] = rc[:Y_ATTN].reshape(R, S)
        den_flat[c * S:(c + 1) * S] = rc[Y_ATTN:].view(np.float32)
    attn_rT[:R] /= den_flat[None, :]
    attn_rT[R] = 1.0
    # bias folded into the GEMM via the ones row
    wo33 = np.concatenate(
        [np.asarray(Wo, np.float32),
         np.asarray(bo, np.float32)[None, :]], axis=0)   # [R+1, D]
    out = np.empty((B * S, D), np.float32)
    # attn_rT.T is F-contiguous: BLAS consumes it without a copy
    np.dot(attn_rT.T, np.ascontiguousarray(wo33), out=out)
    t4 = _time.time()
    if _DBG:
        print(
            f"[kernel] runner {t1-t0:.3f}s  prep {t2-t1:.3f}s  "
            f"dev {t3-t2:.3f}s  proj {t4-t3:.3f}s  TOTAL {t4-t0:.3f}s",
            file=sys.stderr,
        )
    return out.reshape(B, S, D)


if __name__ == "__main__":
    rng = np.random.default_rng(0)
    x = rng.standard_normal((B, S, D), dtype=np.float32)
    s_in, s_r = 1.0 / np.sqrt(D), 1.0 / np.sqrt(R)
    mk = lambda sh, s: rng.uniform(-s, s, sh).astype(np.float32)
    out = kernel(x, mk((D, R), s_in), mk((R,), s_in), mk((D, R), s_in),
                 mk((R,), s_in), mk((D, R), s_in), mk((R,), s_in),
                 mk((R, D), s_r), mk((D,), s_r))
    print("ran ok", out.shape, out[0, 0, :4])


# revision 26
# speedup vs baseline: 1.3853x; 1.3853x over previous
"""Low-rank self-attention Trainium2 kernel — tunnel-optimized split.

The axon tunnel to the NeuronCores moves ~30-100 MB/s with ~0.1s fixed
cost per transfer, while the rank-32 projections are ~4 GFLOP of host BLAS
(~0.1s). So the host computes QKV = x@Wqkv+b (f32) and the final
attn@Wo+bo projection, and the device runs only the S^2 attention core
(the dominant FLOPs): scoresT = K^T.T @ Q^T (rank-32 contraction, 4-way
row-packed fp32r), expS = exp(scale*scores) on ACT, and attn^T[33,q]
accumulated over 32 k-tiles with a fused ones-column denominator.

Sharding: one batch per core on 4 of the 8 cores — full queries per core,
so K/V are never duplicated across cores (pairs of query-half cores would
each need the full K/V). Per-core traffic: one packed bf16 input blob
(Q^T/K^T [32,4096] + V [4096,32] = 768KB) and one packed bf16 output
(attn^T [32,4096] + den f32-as-bf16-pairs = 272KB).

The jitted shard_map callable is built once and cached; donated output
buffers are created on-device (zeros_fn).
"""
import os
import sys

sys.path.insert(0, "/opt/trn_rl_repo")

import numpy as np
import ml_dtypes

import concourse.bass as bass
import concourse.mybir as mybir
import concourse.tile as tile
from bass_rust import ScopedClock

BF16 = mybir.dt.bfloat16
F32 = mybir.dt.float32
F32R = mybir.dt.float32r

B, S, D, R = 4, 4096, 1024, 32
N_CORES = 4
SCALE = float(R) ** -0.5

QT_OFF = 0
KT_OFF = QT_OFF + R * S           # 131072
V_OFF = KT_OFF + R * S            # 262144 (V natural [S, R])
BLOB = V_OFF + S * R              # 393216 bf16 elems (768 KiB)
Y_ATTN = R * S                    # 131072
Y_ELEMS = Y_ATTN + 2 * S          # + den as f32 bitcast into bf16 pairs

_DBG = bool(os.environ.get("KERNEL_DEBUG_TIMING"))


class ChunkedDrainTileContext(tile.TileContext):
    """This walrus build rejects >1 sync wait on the kernel-tail drain;
    spread the final drain's waits across single-wait SP nops."""

    def _drain_and_barrier(self, tick_clock, wait_clock):
        nc = self.nc
        MAX_NOPS = 40
        nops = [nc.sync.nop(nofuse=True) for _ in range(MAX_NOPS)]
        drain_inst = nc.sync.drain()
        wait_clock.add_sem_waits(
            drain_inst.ins, ScopedClock({None: tick_clock.global_clock})
        )
        si = drain_inst.ins.sync_info
        waits = list(si.on_wait) if si and si.on_wait else []
        if len(waits) > 1:
            assert len(waits) <= 1 + MAX_NOPS, f"too many drain waits: {len(waits)}"
            drain_inst.ins.sync_info = mybir.SyncInfo(
                on_wait=[waits[0]], on_update=si.on_update
            )
            for i, w in enumerate(waits[1:]):
                nop = nops[i]
                old = nop.ins.sync_info
                nop.ins.sync_info = mybir.SyncInfo(
                    on_wait=[w], on_update=old.on_update if old else []
                )
        nc.all_engine_barrier()
        assert self.sems is not None
        popped = nc._tile_sem_poison_stack.pop()
        assert popped is self._sem_poison
        nc.clear_and_free_semaphores(list(self.sems.allocated().values()))
        nc.all_engine_barrier()
        split_multi_waits(nc)


def split_multi_waits(nc):
    """walrus in this container rejects instructions with more than one sync
    wait; split extras onto same-engine nops placed immediately before."""
    for f in nc.m.functions:
        for bb in f.blocks:
            snap = list(bb.instructions)
            if not any(
                inst.sync_info and inst.sync_info.on_wait
                and len(inst.sync_info.on_wait) > 1
                for inst in snap
            ):
                continue
            newlist = []
            created = set()
            for inst in snap:
                si = inst.sync_info
                waits = list(si.on_wait) if si and si.on_wait else []
                if len(waits) > 1:
                    eng = inst.engine
                    for w in waits[:-1]:
                        nop = nc.engines[eng].nop(nofuse=True)
                        nop.ins.sync_info = mybir.SyncInfo(
                            on_wait=[w], on_update=[]
                        )
                        created.add(nop.ins.name)
                        newlist.append(nop.ins)
                    inst.sync_info = mybir.SyncInfo(
                        on_wait=[waits[-1]], on_update=si.on_update
                    )
                newlist.append(inst)
            # nops were auto-appended to the current bb; strip strays
            for f2 in nc.m.functions:
                for bb2 in f2.blocks:
                    if bb2 is bb:
                        continue
                    cur = list(bb2.instructions)
                    if any(i.name in created for i in cur):
                        bb2.instructions = [
                            i for i in cur if i.name not in created
                        ]
            tail = [i for i in bb.instructions if i.name in created
                    and i not in snap]
            seen = set()
            final = []
            for i in newlist:
                if i.name in seen:
                    continue
                seen.add(i.name)
                final.append(i)
            bb.instructions = final


def build_kernel():
    nc = bass.Bass("TRN2", target_bir_lowering=False, debug=False)

    blob = nc.dram_tensor("blob", [BLOB], BF16, kind="ExternalInput")
    y = nc.dram_tensor("y", [Y_ELEMS], BF16, kind="ExternalOutput")

    NKT = S // 128   # 32 k-tiles
    NQC = S // 512   # 8 query chunks
    Exp = mybir.ActivationFunctionType.Exp

    with ChunkedDrainTileContext(nc) as tc:
        with (
            tc.tile_pool(name="persist", bufs=1) as pp,
            tc.tile_pool(name="work", bufs=3) as wp,
            tc.tile_pool(name="expp", bufs=2) as ep,
            tc.tile_pool(name="psB", bufs=1, space="PSUM") as psB,
            tc.tile_pool(name="psB2", bufs=2, space="PSUM") as psB2,
        ):
            qT16 = pp.tile([R, S], BF16)
            nc.sync.dma_start(
                qT16[:],
                blob.ap()[QT_OFF:QT_OFF + R * S].rearrange("(r s) -> r s", s=S),
            )
            kT16 = pp.tile([R, S], BF16)
            nc.sync.dma_start(
                kT16[:],
                blob.ap()[KT_OFF:KT_OFF + R * S].rearrange("(r s) -> r s", s=S),
            )
            vone = pp.tile([128, NKT, 33], BF16)
            nc.sync.dma_start(
                vone[:, :, 0:32],
                blob.ap()[V_OFF:V_OFF + S * R].rearrange(
                    "(kt p r) -> p kt r", p=128, r=R
                ),
            )
            nc.vector.memset(vone[:, :, 32], 1.0)

            qTf = pp.tile([R, S], F32R)
            nc.vector.tensor_copy(out=qTf[:], in_=qT16[:])
            kTf = pp.tile([R, S], F32R)
            nc.vector.tensor_copy(out=kTf[:], in_=kT16[:])
            qT_rep = pp.tile([128, S], F32R)
            kT_rep = pp.tile([128, S], F32R)
            for i in range(4):
                nc.sync.dma_start(qT_rep[32 * i:32 * i + 32, :], qTf[:])
                nc.sync.dma_start(kT_rep[32 * i:32 * i + 32, :], kTf[:])

            y2d = y.ap()[0:Y_ATTN].rearrange("(r s) -> r s", s=S)
            for qc in range(NQC):
                expT = ep.tile([128, NKT, 512], BF16, tag="expT")
                for g in range(NKT // 4):
                    ps_s = psB.tile([128, 4, 512], F32, tag="ps_s")
                    for i in range(4):
                        kt = g * 4 + i
                        nc.tensor.matmul(
                            ps_s[:, i, :],
                            kT_rep[32 * i:32 * i + 32,
                                   kt * 128:(kt + 1) * 128],
                            qT_rep[32 * i:32 * i + 32,
                                   qc * 512:(qc + 1) * 512],
                            start=True, stop=True,
                            skip_group_check=True,
                            tile_position=(32 * i, 0),
                        )
                    nc.scalar.activation(
                        expT[:, g * 4:(g + 1) * 4, :], ps_s[:], Exp,
                        scale=SCALE,
                    )
                pa = psB2.tile([128, 512], F32, tag="pa")
                for kt in range(NKT):
                    nc.tensor.matmul(
                        pa[0:33, :], vone[:, kt, :], expT[:, kt, :],
                        start=(kt == 0), stop=(kt == NKT - 1),
                    )
                attn16 = wp.tile([R, 512], BF16, tag="attn16")
                nc.vector.tensor_copy(out=attn16[:], in_=pa[0:32, :])
                den32 = wp.tile([1, 512], F32, tag="den32")
                nc.vector.tensor_copy(out=den32[:], in_=pa[32:33, :])
                nc.sync.dma_start(
                    y2d[:, qc * 512:(qc + 1) * 512], attn16[:]
                )
                nc.sync.dma_start(
                    y.ap()[Y_ATTN + qc * 1024:Y_ATTN + (qc + 1) * 1024]
                    .rearrange("(a b) -> a b", a=1),
                    den32[:].bitcast(BF16),
                )
    return nc


_CACHE = {}


def _get_runner():
    if "runner" in _CACHE:
        return _CACHE["runner"]
    import time as _time

    t0 = _time.time()
    import jax
    import jax.numpy as jnp
    from jax.experimental.shard_map import shard_map
    from jax.sharding import Mesh, NamedSharding, PartitionSpec

    from concourse import bass2jax

    bass2jax.install_neuronx_cc_hook()
    nc = build_kernel()

    out_aval = jax.core.ShapedArray((Y_ELEMS,), ml_dtypes.bfloat16)
    partition_name = (
        nc.partition_id_tensor.name if nc.partition_id_tensor else None
    )
    in_names = ("blob", "y") + ((partition_name,) if partition_name else ())

    def _body(blob_arg, yzero):
        operands = [blob_arg, yzero]
        if partition_name is not None:
            operands.append(bass2jax.partition_id_tensor())
        outs = bass2jax._bass_exec_p.bind(
            *operands,
            out_avals=(out_aval,),
            in_names=in_names,
            out_names=("y",),
            lowering_input_output_aliases=(),
            sim_require_finite=True,
            sim_require_nnan=True,
            nc=nc,
        )
        return tuple(outs)

    devices = jax.devices()[:N_CORES]
    mesh = Mesh(np.asarray(devices), ("core",))
    p = PartitionSpec("core")
    sharded = jax.jit(
        shard_map(_body, mesh=mesh, in_specs=(p, p), out_specs=(p,),
                  check_rep=False),
        donate_argnums=(1,), keep_unused=True,
    )
    zsh = NamedSharding(mesh, p)
    zeros_fn = jax.jit(
        lambda: jnp.zeros((N_CORES * Y_ELEMS,), ml_dtypes.bfloat16),
        out_shardings=zsh,
    )
    blob_sh = NamedSharding(mesh, p)
    runner = (sharded, zeros_fn, jax, devices, blob_sh)
    _CACHE["runner"] = runner
    if _DBG:
        print(f"[kernel] runner built in {_time.time()-t0:.1f}s",
              file=sys.stderr)
    return runner


def kernel(x, Wq, bq, Wk, bk, Wv, bv, Wo, bo):
    import time as _time

    t0 = _time.time()
    sharded, zeros_fn, jax, devices, blob_sh = _get_runner()
    t1 = _time.time()
    yzero = zeros_fn()  # async; on-device, donated below

    bf16 = ml_dtypes.bfloat16
    x = np.ascontiguousarray(np.asarray(x, np.float32)).reshape(B * S, D)
    wqkv = np.concatenate(
        [np.asarray(Wq, np.float32), np.asarray(Wk, np.float32),
         np.asarray(Wv, np.float32)], axis=1)            # [D, 96]
    bqkv = np.concatenate(
        [np.asarray(bq, np.float32), np.asarray(bk, np.float32),
         np.asarray(bv, np.float32)])                    # [96]

    # Per-batch pipeline: GEMM + pack core c, then start its H2D while the
    # next batch's GEMM runs — overlaps host BLAS with tunnel transfer.
    dev_shards = []
    qkv_b = np.empty((S, 3 * R), np.float32)
    for c in range(N_CORES):
        np.dot(x[c * S:(c + 1) * S], wqkv, out=qkv_b)
        qkv_b += bqkv
        q16 = qkv_b.astype(bf16)                         # [S, 96]
        bl = np.empty((BLOB,), bf16)
        bl[QT_OFF:QT_OFF + R * S] = q16[:, 0:R].T.reshape(-1)
        bl[KT_OFF:KT_OFF + R * S] = q16[:, R:2 * R].T.reshape(-1)
        bl[V_OFF:V_OFF + S * R] = q16[:, 2 * R:3 * R].reshape(-1)
        dev_shards.append(jax.device_put(bl, devices[c]))  # async H2D
    blob_arr = jax.make_array_from_single_device_arrays(
        (N_CORES * BLOB,), blob_sh, dev_shards)
    t2 = _time.time()

    out_arrs = sharded(blob_arr, yzero)
    shards = sorted(out_arrs[0].addressable_shards,
                    key=lambda s: s.index[0].start or 0)
    datas = [s.data for s in shards]
    for d in datas:
        d.copy_to_host_async()                           # parallel D2H
    # bias folded into the GEMM via a ones row
    wo33 = np.ascontiguousarray(np.concatenate(
        [np.asarray(Wo, np.float32),
         np.asarray(bo, np.float32)[None, :]], axis=0))  # [R+1, D]
    out = np.empty((B * S, D), np.float32)
    a_rT = np.empty((R + 1, S), np.float32)              # rank-major + ones
    a_rT[R] = 1.0
    for c in range(N_CORES):
        rc = np.asarray(datas[c])        # blocks on shard c only; later
        a_rT[:R] = rc[:Y_ATTN].reshape(R, S)   # shards keep streaming
        a_rT[:R] /= rc[Y_ATTN:].view(np.float32)[None, :]
        # a_rT.T is F-contiguous: BLAS consumes it without a copy
        np.dot(a_rT.T, wo33, out=out[c * S:(c + 1) * S])
    t3 = _time.time()
    if _DBG:
        print(
            f"[kernel] runner {t1-t0:.3f}s  prep {t2-t1:.3f}s  "
            f"dev+proj {t3-t2:.3f}s  TOTAL {t3-t0:.3f}s",
            file=sys.stderr,
        )
    return out.reshape(B, S, D)


if __name__ == "__main__":
    rng = np.random.default_rng(0)
    x = rng.standard_normal((B, S, D), dtype=np.float32)
    s_in, s_r = 1.0 / np.sqrt(D), 1.0 / np.sqrt(R)
    mk = lambda sh, s: rng.uniform(-s, s, sh).astype(np.float32)
    out = kernel(x, mk((D, R), s_in), mk((R,), s_in), mk((D, R), s_in),
                 mk((R,), s_in), mk((D, R), s_in), mk((R,), s_in),
                 mk((R, D), s_r), mk((D,), s_r))
    print("ran ok", out.shape, out[0, 0, :4])
